# revision 19
# baseline (speedup 1.0000x reference)
"""Trainium2 Bass kernel for AMM (landmark/Nystrom-style) attention.

Per batch element (8 total, one NeuronCore each):
    qkv  = x @ W_qkv; q,k,v = split(qkv); q /= sqrt(512)
    keys_lm = segment_mean(k, 16); vals_lm = segment_mean(v, 16)
    out  = softmax(q @ keys_lm^T) @ vals_lm @ W_proj + b_proj
    return v + out

Algebraic restructuring (exact in real arithmetic):
  - segment_mean commutes with the projections: keys_lm = pool(x) @ W_k,
    vals_lm = pool(x) @ W_v; pool(x) is computed on the host (16M adds vs
    17G device MACs) so the landmark chain starts ~2us into the kernel.
  - the q projection is never computed: scores = x @ (W_q @ keys_lm^T),
    and W_q @ keys_lm^T is only 512x256, once per core.
  - attn @ vals_lm @ W_proj -> attn @ (vals_lm @ W_proj + 1 b_proj) = VWb
    (b_proj folded via softmax rows summing to 1).
  - softmax normalization applied BEFORE the value matmul:
        attn = E * (1/den) with den broadcast to all partitions by a
    GpSimd partition_all_reduce, so the value matmul emits the final
    attention output directly.
  - THE RESIDUAL ADD IS FREE: the v projection (x @ W_v) and the value
    matmul (attn @ VWb) accumulate into the SAME PSUM tile; PSUM holds
    v + attn@VWb and the only post-op is a psum->sbuf copy (DVE, bf16)
    followed by the output DMA. No scalar_tensor_tensor, no v staging.

Engine layout: PE does all matmuls (v / landmarks / scores / out2) nearly
back-to-back; ACT does landmark copies then exps (one activation-table
switch); GpSimd does the den partition-reductions; DVE does den-sum,
reciprocal, attn scaling and the psum evictions. Output is bf16 (host
upcasts); input DMAs are chunk-major single-instruction transfers so the
HWDGE generator is never the bottleneck.

Sharding: pure data-parallel over batch B=8 across 8 cores, weights
replicated, no collectives. Host pre-transposes x per core, pre-pools the
landmark means, and casts matmul inputs to bf16 (fp32 PSUM accumulation).
"""

import sys
from contextlib import ExitStack

import numpy as np

sys.path.insert(0, "/opt/trn_rl_repo")

import concourse.bass as bass  # noqa: E402
import concourse.tile as tile  # noqa: E402
from concourse import bacc, bass_isa, mybir  # noqa: E402
from concourse.bass_utils import run_bass_kernel_spmd  # noqa: E402

import ml_dtypes  # noqa: E402

BF16 = mybir.dt.bfloat16
F32 = mybir.dt.float32
AF = mybir.ActivationFunctionType
ALU = mybir.AluOpType

B, N, DIM = 8, 4096, 512
L, SEG = 256, 16
CT = DIM // 128
MT = N // 512
XCH = 512
NCH = N // XCH

RSCALE = float(1.0 / np.sqrt(512.0))


def build_kernel(ctx: ExitStack, tc: "tile.TileContext", out_d, xt_d, wkv_d, xpool_d, wqT_d, wproj_d, bproj_d):
    nc = tc.nc

    consts = ctx.enter_context(tc.tile_pool(name="consts", bufs=1))
    work = ctx.enter_context(tc.tile_pool(name="work", bufs=2))
    pm = ctx.enter_context(tc.tile_pool(name="pm", bufs=4, space="PSUM"))
    po = ctx.enter_context(tc.tile_pool(name="po", bufs=4, space="PSUM"))

    wkv = consts.tile([128, CT, 2 * DIM], BF16)  # [c_lo, cj, (k|v) columns]
    xt = consts.tile([128, CT, N], BF16)
    xpool = consts.tile([128, CT, L], BF16)
    wqT = consts.tile([128, CT, DIM], BF16)
    wproj = consts.tile([128, CT, DIM], BF16)
    bproj = consts.tile([1, DIM], BF16)
    ones_row = consts.tile([1, 128], BF16)

    # Input DMAs spread over three HWDGE queues so the first transfers
    # overlap; order within each queue matches the PE program order
    # keysT -> kw -> scores(0) -> valsT -> scores(1) -> vw -> v.
    nc.sync.dma_start(out=xpool[:, :, :], in_=xpool_d[:, :, :])
    nc.sync.dma_start(out=wkv[:, :, 0:DIM], in_=wkv_d[1])
    nc.sync.dma_start(out=wqT[:, :, :], in_=wqT_d[:, :, :])
    nc.sync.dma_start(out=xt[:, :, 0:XCH], in_=xt_d[0])
    nc.sync.dma_start(out=wkv[:, :, DIM : 2 * DIM], in_=wkv_d[0])
    nc.sync.dma_start(out=xt[:, :, XCH : 2 * XCH], in_=xt_d[1])
    nc.sync.dma_start(out=wproj[:, :, :], in_=wproj_d[:, :, :])
    nc.sync.dma_start(out=bproj[:, :], in_=bproj_d[:, :])
    for ci in range(2, NCH):
        nc.sync.dma_start(out=xt[:, :, ci * XCH : (ci + 1) * XCH], in_=xt_d[ci])

    # PE clock warm-up: the cost of a PE idle gap includes a ~3us re-ramp
    # from 0.65GHz. Run junk matmuls on a memset scratch tile while the
    # first DMAs land so the real matmuls start at full clock.
    scratch = consts.tile([128, 512], BF16)
    nc.gpsimd.memset(scratch[:, :], 0.0)
    nc.vector.memset(ones_row[:, :], 1.0)
    for w in range(8):
        wp = pm.tile([128, 512], F32, tag="mm", name="warm")
        nc.tensor.matmul(
            wp[:, :], scratch[:, 0:128], scratch[:, :], start=True, stop=True
        )

    keysT = consts.tile([128, CT, L], BF16)
    valsT = consts.tile([128, CT, L], BF16)
    kw = consts.tile([128, CT, L], BF16)
    vw = consts.tile([128, 2, DIM], BF16)

    out_ps = {}

    def v_group(t):
        # Opens the output psum tile for rows [t*128, (t+1)*128): accumulates
        # x @ W_v; the attention matmuls later close the group.
        op = po.tile([128, 512], F32, tag="out", name=f"op{t}")
        r0 = t * 128
        for cj in range(CT):
            nc.tensor.matmul(
                op[:, :],
                xt[:, cj, r0 : r0 + 128],
                wkv[:, cj, DIM : 2 * DIM],
                start=(cj == 0),
                stop=False,
                skip_group_check=True,
            )
        return op

    def lm_proj(dst, col0):
        for dj in range(CT):
            pt = pm.tile([128, L], F32, tag="mm", name="ptl")
            for cj in range(CT):
                nc.tensor.matmul(
                    pt[:, :],
                    wkv[:, cj, col0 + dj * 128 : col0 + (dj + 1) * 128],
                    xpool[:, cj, :],
                    start=(cj == 0),
                    stop=(cj == CT - 1),
                )
            nc.scalar.copy(dst[:, dj, :], pt[:, :])

    def scores_p1(mi):
        # scores -> exp -> den (gpsimd partition reduce, replicated to all
        # partitions) -> den0+den1 (gpsimd). The DVE half is scores_p2, issued
        # later so DVE's in-order queue never blocks the psum evictions.
        et = work.tile([128, 2, 512], BF16, tag="et", bufs=3, name="et")
        for li in range(2):
            pt = pm.tile([128, 512], F32, tag="mm", name="pts")
            for cj in range(CT):
                nc.tensor.matmul(
                    pt[:, :],
                    kw[:, cj, li * 128 : (li + 1) * 128],
                    xt[:, cj, mi * 512 : (mi + 1) * 512],
                    start=(cj == 0),
                    stop=(cj == CT - 1),
                )
            nc.scalar.activation(et[:, li, :], pt[:, :], AF.Exp, scale=RSCALE)
        den = work.tile([128, 2, 512], F32, tag="den", bufs=2, name="den")
        for li in range(2):
            nc.gpsimd.partition_all_reduce(
                den[:, li, :], et[:, li, :], 128, bass_isa.ReduceOp.add
            )
        dsum = work.tile([128, 512], F32, tag="dsum", bufs=2, name="dsum")
        nc.gpsimd.tensor_tensor(dsum[:, :], den[:, 0, :], den[:, 1, :], ALU.add)
        return et, dsum

    def scores_p2(et, dsum):
        rr = work.tile([128, 512], BF16, tag="rr", bufs=2, name="rr")
        attn = work.tile([128, 2, 512], BF16, tag="attn", bufs=3, name="attn")
        with nc.allow_low_precision(reason="1/den and attn weights tolerate bf16"):
            nc.vector.reciprocal(rr[:, :], dsum[:, :])
            for li in range(2):
                nc.vector.tensor_tensor(
                    attn[:, li, :], et[:, li, :], rr[:, :], ALU.mult
                )
        return attn

    # ---- pipeline fill: landmark chain interleaved with scores(0|1) --------
    lm_proj(keysT, 0)
    for cj in range(CT):
        pt = pm.tile([128, L], F32, tag="mm", name="ptk")
        for dj in range(CT):
            nc.tensor.matmul(
                pt[:, :],
                wqT[:, dj, cj * 128 : (cj + 1) * 128],
                keysT[:, dj, :],
                start=(dj == 0),
                stop=(dj == CT - 1),
            )
        nc.scalar.copy(kw[:, cj, :], pt[:, :])
    p1_0 = scores_p1(0)
    lm_proj(valsT, DIM)
    p1_1 = scores_p1(1)
    for li in range(2):
        pt = pm.tile([128, DIM], F32, tag="mm", name="ptv")
        for dj in range(CT):
            nc.tensor.matmul(
                pt[:, :],
                valsT[:, dj, li * 128 : (li + 1) * 128],
                wproj[:, dj, :],
                start=(dj == 0),
                stop=False,
            )
        nc.tensor.matmul(pt[:, :], ones_row[:, :], bproj[:, :], start=False, stop=True)
        nc.scalar.copy(vw[:, li, :], pt[:, :])

    for t in range(4):
        out_ps[t] = v_group(t)
    attn_q = [scores_p2(*p1_0), scores_p2(*p1_1)]

    # ---- main loop: close out tiles with attn @ VWb, evict, stream ---------
    # scores_p1(mi+2) (PE/ACT/gpsimd) leads the block; its DVE half
    # (scores_p2) trails the block, behind this block's eviction copies.
    for mi in range(MT):
        p1 = scores_p1(mi + 2) if mi + 2 < MT else None
        attn = attn_q.pop(0)
        osb = work.tile([128, 4, 512], BF16, tag="osb", bufs=2, name="osb")
        # In the last two blocks ACT has no exps left: split each eviction
        # ACT/DVE half-and-half (one Copy-table switch, off the critical
        # path) and DMA per tile so the kernel tail drains ~2x faster.
        tail = mi >= MT - 2
        r0 = mi * 512
        for t in range(4):
            op = out_ps.pop(mi * 4 + t)
            sl = slice(t * 128, (t + 1) * 128)
            for li in range(2):
                nc.tensor.matmul(
                    op[:, :],
                    attn[:, li, sl],
                    vw[:, li, :],
                    start=False,
                    stop=(li == 1),
                    skip_group_check=True,
                )
            if tail and t % 2 == 0:
                nc.scalar.copy(osb[:, t, :], op[:, :])
            else:
                nc.vector.tensor_copy(osb[:, t, :], op[:, :])
        if tail:
            for h, q in ((0, nc.sync), (1, nc.scalar)):
                q.dma_start(
                    out=out_d[r0 + h * 256 : r0 + (h + 1) * 256, :].rearrange(
                        "(t p) d -> p t d", p=128
                    ),
                    in_=osb[:, 2 * h : 2 * h + 2, :],
                )
        else:
            nc.sync.dma_start(
                out=out_d[r0 : r0 + 512, :].rearrange("(t p) d -> p t d", p=128),
                in_=osb[:, :, :],
            )
        if mi + 1 < MT:
            for t in range(4):
                out_ps[(mi + 1) * 4 + t] = v_group((mi + 1) * 4 + t)
        if p1 is not None:
            attn_q.append(scores_p2(*p1))


def build_nc(repeat: int = 1):
    nc = bacc.Bacc("TRN2", target_bir_lowering=False, debug=False, num_devices=8)
    xt_d = nc.declare_dram_parameter("xt", [NCH, 128, CT, XCH], BF16, isOutput=False)
    wkv_d = nc.declare_dram_parameter("wkv", [2, 128, CT, DIM], BF16, isOutput=False)
    xpool_d = nc.declare_dram_parameter("xpool", [128, CT, L], BF16, isOutput=False)
    wqT_d = nc.declare_dram_parameter("wqT", [128, CT, DIM], BF16, isOutput=False)
    wproj_d = nc.declare_dram_parameter("wproj", [128, CT, DIM], BF16, isOutput=False)
    bproj_d = nc.declare_dram_parameter("bproj", [1, DIM], BF16, isOutput=False)
    out_d = nc.declare_dram_parameter("out", [N, DIM], BF16, isOutput=True)
    aps = (
        out_d.ap(),
        xt_d.ap(),
        wkv_d.ap(),
        xpool_d.ap(),
        wqT_d.ap(),
        wproj_d.ap(),
        bproj_d.ap(),
    )
    with tile.TileContext(nc) as tc, ExitStack() as ctx:
        if repeat == 1:
            build_kernel(ctx, tc, *aps)
        else:
            with tc.For_i(0, repeat, 1):
                build_kernel(ctx, tc, *aps)
    nc.compile()
    return nc


def prep_in_maps(x, W_qkv, W_proj, b_proj):
    bf = ml_dtypes.bfloat16
    W = np.asarray(W_qkv, np.float32)
    kv = W[:, DIM:].reshape(CT, 128, 2, DIM)  # [cj, c_lo, (k,v), col]
    wkv = np.ascontiguousarray(
        np.stack([kv[:, :, 1, :], kv[:, :, 0, :]], axis=0).transpose(0, 2, 1, 3)
    ).astype(bf)  # [v|k, c_lo, cj, col]
    wqT = np.ascontiguousarray(
        W[:, :DIM].T.reshape(CT, 128, DIM).transpose(1, 0, 2)
    ).astype(bf)  # [d_lo, dj, c]
    wp = np.ascontiguousarray(
        np.asarray(W_proj, np.float32).reshape(CT, 128, DIM).transpose(1, 0, 2)
    ).astype(bf)  # [d_lo, dj, dcol]
    bp = np.asarray(b_proj, np.float32).astype(bf).reshape(1, DIM)
    in_maps = []
    for i in range(B):
        xi = np.asarray(x[i], np.float32)
        xT = xi.T  # [c, n]
        xt = np.ascontiguousarray(
            xT.reshape(CT, 128, NCH, XCH).transpose(2, 1, 0, 3)
        ).astype(bf)  # [ci, c_lo, cj, col]
        xp = xi.reshape(L, SEG, DIM).mean(axis=1)  # [l, c]
        xpool = np.ascontiguousarray(
            xp.T.reshape(CT, 128, L).transpose(1, 0, 2)
        ).astype(bf)  # [c_lo, cj, l]
        in_maps.append(
            {"xt": xt, "wkv": wkv, "xpool": xpool, "wqT": wqT, "wproj": wp, "bproj": bp}
        )
    return in_maps


_NC_CACHE = None


def kernel(x, W_qkv, W_proj, b_proj):
    global _NC_CACHE
    if _NC_CACHE is None:
        _NC_CACHE = build_nc()
    nc = _NC_CACHE
    in_maps = prep_in_maps(x, W_qkv, W_proj, b_proj)
    res = run_bass_kernel_spmd(nc, in_maps, core_ids=list(range(B)))
    out = np.stack([res.results[i]["out"] for i in range(B)], axis=0)
    return out.astype(np.float32)


# revision 28
# speedup vs baseline: 1.8842x; 1.8842x over previous
"""Trainium2 Bass kernel for AMM (landmark/Nystrom-style) attention.

Per batch element (8 total, one NeuronCore each):
    qkv  = x @ W_qkv; q,k,v = split(qkv); q /= sqrt(512)
    keys_lm = segment_mean(k, 16); vals_lm = segment_mean(v, 16)
    out  = softmax(q @ keys_lm^T) @ vals_lm @ W_proj + b_proj
    return v + out

Algebraic restructuring (exact in real arithmetic):
  - segment_mean commutes with the projections: keys_lm = pool(x) @ W_k,
    vals_lm = pool(x) @ W_v; pool(x) is computed on the host (16M adds vs
    17G device MACs) so the landmark chain starts ~2us into the kernel.
  - the q projection is never computed: scores = x @ (W_q @ keys_lm^T),
    and W_q @ keys_lm^T is only 512x256, once per core.
  - attn @ vals_lm @ W_proj -> attn @ (vals_lm @ W_proj + 1 b_proj) = VWb
    (b_proj folded via softmax rows summing to 1).
  - softmax normalization applied BEFORE the value matmul:
        attn = E * (1/den) with den broadcast to all partitions by a
    GpSimd partition_all_reduce, so the value matmul emits the final
    attention output directly.
  - THE RESIDUAL ADD IS FREE: the v projection (x @ W_v) and the value
    matmul (attn @ VWb) accumulate into the SAME PSUM tile; PSUM holds
    v + attn@VWb and the only post-op is a psum->sbuf copy (DVE, bf16)
    followed by the output DMA. No scalar_tensor_tensor, no v staging.

Engine layout: PE does all matmuls (v / landmarks / scores / out2) nearly
back-to-back; ACT does landmark copies then exps (one activation-table
switch); GpSimd does the den partition-reductions; DVE does den-sum,
reciprocal, attn scaling and the psum evictions. Output is bf16 (host
upcasts); input DMAs are chunk-major single-instruction transfers so the
HWDGE generator is never the bottleneck.

Sharding: pure data-parallel over batch B=8 across 8 cores, weights
replicated, no collectives. Host pre-transposes x per core, pre-pools the
landmark means, and casts matmul inputs to bf16 (fp32 PSUM accumulation).
"""

import sys
from contextlib import ExitStack

import numpy as np

sys.path.insert(0, "/opt/trn_rl_repo")

import concourse.bass as bass  # noqa: E402
import concourse.tile as tile  # noqa: E402
from concourse import bacc, bass_isa, mybir  # noqa: E402
from concourse.bass_utils import run_bass_kernel_spmd  # noqa: E402

import ml_dtypes  # noqa: E402

BF16 = mybir.dt.bfloat16
F32 = mybir.dt.float32
AF = mybir.ActivationFunctionType
ALU = mybir.AluOpType

B, N, DIM = 8, 4096, 512
L, SEG = 256, 16
CT = DIM // 128
MT = N // 512
XCH = 512
NCH = N // XCH

RSCALE = float(1.0 / np.sqrt(512.0))


def build_kernel(ctx: ExitStack, tc: "tile.TileContext", out_d, xt_d, wkv_d, xpool_d, wqT_d, wproj_d, bproj_d):
    nc = tc.nc

    consts = ctx.enter_context(tc.tile_pool(name="consts", bufs=1))
    work = ctx.enter_context(tc.tile_pool(name="work", bufs=2))
    pm = ctx.enter_context(tc.tile_pool(name="pm", bufs=3, space="PSUM"))
    po = ctx.enter_context(tc.tile_pool(name="po", bufs=4, space="PSUM"))
    pd = ctx.enter_context(tc.tile_pool(name="pd", bufs=1, space="PSUM"))

    wkv = consts.tile([128, CT, 2 * DIM], BF16)  # [c_lo, cj, (k|v) columns]
    xt = consts.tile([128, CT, N], BF16)
    xpool = consts.tile([128, CT, L], BF16)
    wqT = consts.tile([128, CT, DIM], BF16)
    wproj = consts.tile([128, CT, DIM], BF16)
    bproj = consts.tile([1, DIM], BF16)
    ones_row = consts.tile([1, 128], BF16)

    # Input DMAs spread over three HWDGE queues so the first transfers
    # overlap; order within each queue matches the PE program order
    # keysT -> kw -> scores(0) -> valsT -> scores(1) -> vw -> v.
    nc.sync.dma_start(out=xpool[:, :, :], in_=xpool_d[:, :, :])
    nc.sync.dma_start(out=wkv[:, :, 0:DIM], in_=wkv_d[1])
    nc.sync.dma_start(out=wqT[:, :, :], in_=wqT_d[:, :, :])
    nc.sync.dma_start(out=xt[:, :, 0:XCH], in_=xt_d[0])
    nc.sync.dma_start(out=wkv[:, :, DIM : 2 * DIM], in_=wkv_d[0])
    nc.sync.dma_start(out=xt[:, :, XCH : 2 * XCH], in_=xt_d[1])
    nc.sync.dma_start(out=wproj[:, :, :], in_=wproj_d[:, :, :])
    nc.sync.dma_start(out=bproj[:, :], in_=bproj_d[:, :])
    for ci in range(2, NCH):
        nc.sync.dma_start(out=xt[:, :, ci * XCH : (ci + 1) * XCH], in_=xt_d[ci])

    # PE clock warm-up: the cost of a PE idle gap includes a ~3us re-ramp
    # from 0.65GHz. Run junk matmuls on a memset scratch tile while the
    # first DMAs land so the real matmuls start at full clock.
    scratch = consts.tile([128, 512], BF16)
    ones_col = consts.tile([128, 1], BF16)
    nc.vector.memset(scratch[:, :], 0.0)
    nc.vector.memset(ones_row[:, :], 1.0)
    nc.vector.memset(ones_col[:, :], 1.0)
    for w in range(8):
        wp = pm.tile([128, 512], F32, tag="mm", name="warm")
        nc.tensor.matmul(
            wp[:, :], scratch[:, 0:128], scratch[:, :], start=True, stop=True
        )

    keysT = consts.tile([128, CT, L], BF16)
    valsT = consts.tile([128, CT, L], BF16)
    kw = consts.tile([128, CT, L], BF16)
    vw = consts.tile([128, 2, DIM], BF16)

    out_ps = {}

    def v_group(t):
        # Opens the output psum tile for rows [t*128, (t+1)*128): accumulates
        # x @ W_v; the attention matmuls later close the group.
        op = po.tile([128, 512], F32, tag="out", name=f"op{t}")
        r0 = t * 128
        for cj in range(CT):
            nc.tensor.matmul(
                op[:, :],
                xt[:, cj, r0 : r0 + 128],
                wkv[:, cj, DIM : 2 * DIM],
                start=(cj == 0),
                stop=False,
                skip_group_check=True,
            )
        return op

    def lm_proj(dst, col0):
        for dj in range(CT):
            pt = pm.tile([128, L], F32, tag="mm", name="ptl")
            for cj in range(CT):
                nc.tensor.matmul(
                    pt[:, :],
                    wkv[:, cj, col0 + dj * 128 : col0 + (dj + 1) * 128],
                    xpool[:, cj, :],
                    start=(cj == 0),
                    stop=(cj == CT - 1),
                )
            nc.scalar.copy(dst[:, dj, :], pt[:, :])

    def scores_p1(mi):
        # scores -> exp. The den/normalize steps are issued separately and
        # later (PE den-row + PE broadcast + DVE recip/scale) so no engine's
        # in-order queue ever blocks the psum evictions.
        et = work.tile([128, 2, 512], BF16, tag="et", bufs=3, name="et")
        for li in range(2):
            pt = pm.tile([128, 512], F32, tag="mm", name="pts")
            for cj in range(CT):
                nc.tensor.matmul(
                    pt[:, :],
                    kw[:, cj, li * 128 : (li + 1) * 128],
                    xt[:, cj, mi * 512 : (mi + 1) * 512],
                    start=(cj == 0),
                    stop=(cj == CT - 1),
                )
            nc.scalar.activation(et[:, li, :], pt[:, :], AF.Exp, scale=RSCALE)
        return et

    def den_row(et):
        # den[1, n] = sum over all 256 landmarks of E: two N=512 matmuls with
        # the all-ones column stationary, accumulating li halves in psum.
        dp = pd.tile([128, 512], F32, tag="den", name="dp")
        for li in range(2):
            nc.tensor.matmul(
                dp[0:1, :],
                ones_col[:, :],
                et[:, li, :],
                start=(li == 0),
                stop=(li == 1),
            )
        rr = work.tile([1, 512], BF16, tag="rr", bufs=2, name="rr")
        return dp, rr

    def recip(dp, rr):
        with nc.allow_low_precision(reason="1/den tolerates bf16"):
            nc.vector.reciprocal(rr[:, :], dp[0:1, :])

    def bcast(rr):
        # Replicate 1/den to all 128 partitions with a K=1 outer-product
        # matmul: rrb = ones_row^T @ rr_row.
        rrb = pm.tile([128, 512], F32, tag="mm", name="rrb")
        nc.tensor.matmul(rrb[:, :], ones_row[:, :], rr[:, :], start=True, stop=True)
        return rrb

    def scales(et, rrb):
        attn = work.tile([128, 2, 512], BF16, tag="attn", bufs=2, name="attn")
        with nc.allow_low_precision(reason="attn weights tolerate bf16"):
            for li in range(2):
                nc.vector.tensor_tensor(
                    attn[:, li, :], et[:, li, :], rrb[:, :], ALU.mult
                )
        return attn

    # ---- pipeline fill: landmark chain interleaved with scores(0|1) --------
    lm_proj(keysT, 0)
    for cj in range(CT):
        pt = pm.tile([128, L], F32, tag="mm", name="ptk")
        for dj in range(CT):
            nc.tensor.matmul(
                pt[:, :],
                wqT[:, dj, cj * 128 : (cj + 1) * 128],
                keysT[:, dj, :],
                start=(dj == 0),
                stop=(dj == CT - 1),
            )
        nc.scalar.copy(kw[:, cj, :], pt[:, :])
    ets = {0: scores_p1(0)}
    lm_proj(valsT, DIM)
    ets[1] = scores_p1(1)
    for li in range(2):
        pt = pm.tile([128, DIM], F32, tag="mm", name="ptv")
        for dj in range(CT):
            nc.tensor.matmul(
                pt[:, :],
                valsT[:, dj, li * 128 : (li + 1) * 128],
                wproj[:, dj, :],
                start=(dj == 0),
                stop=False,
            )
        nc.tensor.matmul(pt[:, :], ones_row[:, :], bproj[:, :], start=False, stop=True)
        nc.scalar.copy(vw[:, li, :], pt[:, :])

    dp0, rr0 = den_row(ets[0])
    recip(dp0, rr0)
    for t in range(4):
        out_ps[t] = v_group(t)
    rrb0 = bcast(rr0)
    attn_q = [scales(ets.pop(0), rrb0)]

    # ---- main loop: close out tiles with attn @ VWb, evict, stream ---------
    # Block order: scores(mi+2) -> den(mi+1) [recip queued ahead of this
    # block's copies on DVE] -> out2(mi)+evictions -> v(mi+1) -> bcast(mi+1)
    # -> scales(mi+1). Each engine's in-order queue stays unblocked.
    for mi in range(MT):
        if mi + 2 < MT:
            ets[mi + 2] = scores_p1(mi + 2)
        nxt = None
        if mi + 1 < MT:
            et_nxt = ets.pop(mi + 1)
            dp, rr = den_row(et_nxt)
            recip(dp, rr)
            nxt = (et_nxt, rr)
        attn = attn_q.pop(0)
        osb = work.tile([128, 4, 512], BF16, tag="osb", bufs=2, name="osb")
        # In the last two blocks ACT has no exps left: split each eviction
        # ACT/DVE half-and-half (one Copy-table switch, off the critical
        # path) and DMA per tile so the kernel tail drains ~2x faster.
        tail = mi >= MT - 2
        r0 = mi * 512
        for t in range(4):
            op = out_ps.pop(mi * 4 + t)
            sl = slice(t * 128, (t + 1) * 128)
            for li in range(2):
                nc.tensor.matmul(
                    op[:, :],
                    attn[:, li, sl],
                    vw[:, li, :],
                    start=False,
                    stop=(li == 1),
                    skip_group_check=True,
                )
            if tail and t % 2 == 0:
                nc.scalar.copy(osb[:, t, :], op[:, :])
            else:
                nc.vector.tensor_copy(osb[:, t, :], op[:, :])
        if tail:
            for h, q in ((0, nc.sync), (1, nc.scalar)):
                q.dma_start(
                    out=out_d[r0 + h * 256 : r0 + (h + 1) * 256, :].rearrange(
                        "(t p) d -> p t d", p=128
                    ),
                    in_=osb[:, 2 * h : 2 * h + 2, :],
                )
        else:
            nc.sync.dma_start(
                out=out_d[r0 : r0 + 512, :].rearrange("(t p) d -> p t d", p=128),
                in_=osb[:, :, :],
            )
        if mi + 1 < MT:
            for t in range(4):
                out_ps[(mi + 1) * 4 + t] = v_group((mi + 1) * 4 + t)
        if nxt is not None:
            et_nxt, rr = nxt
            rrb = bcast(rr)
            attn_q.append(scales(et_nxt, rrb))


def build_nc(repeat: int = 1):
    nc = bacc.Bacc("TRN2", target_bir_lowering=False, debug=False, num_devices=8)
    xt_d = nc.declare_dram_parameter("xt", [NCH, 128, CT, XCH], BF16, isOutput=False)
    wkv_d = nc.declare_dram_parameter("wkv", [2, 128, CT, DIM], BF16, isOutput=False)
    xpool_d = nc.declare_dram_parameter("xpool", [128, CT, L], BF16, isOutput=False)
    wqT_d = nc.declare_dram_parameter("wqT", [128, CT, DIM], BF16, isOutput=False)
    wproj_d = nc.declare_dram_parameter("wproj", [128, CT, DIM], BF16, isOutput=False)
    bproj_d = nc.declare_dram_parameter("bproj", [1, DIM], BF16, isOutput=False)
    out_d = nc.declare_dram_parameter("out", [N, DIM], BF16, isOutput=True)
    aps = (
        out_d.ap(),
        xt_d.ap(),
        wkv_d.ap(),
        xpool_d.ap(),
        wqT_d.ap(),
        wproj_d.ap(),
        bproj_d.ap(),
    )
    with tile.TileContext(nc) as tc, ExitStack() as ctx:
        if repeat == 1:
            build_kernel(ctx, tc, *aps)
        else:
            with tc.For_i(0, repeat, 1):
                build_kernel(ctx, tc, *aps)
    nc.compile()
    return nc


def prep_in_maps(x, W_qkv, W_proj, b_proj):
    bf = ml_dtypes.bfloat16
    W = np.asarray(W_qkv, np.float32)
    kv = W[:, DIM:].reshape(CT, 128, 2, DIM)  # [cj, c_lo, (k,v), col]
    wkv = np.ascontiguousarray(
        np.stack([kv[:, :, 1, :], kv[:, :, 0, :]], axis=0).transpose(0, 2, 1, 3)
    ).astype(bf)  # [v|k, c_lo, cj, col]
    wqT = np.ascontiguousarray(
        W[:, :DIM].T.reshape(CT, 128, DIM).transpose(1, 0, 2)
    ).astype(bf)  # [d_lo, dj, c]
    wp = np.ascontiguousarray(
        np.asarray(W_proj, np.float32).reshape(CT, 128, DIM).transpose(1, 0, 2)
    ).astype(bf)  # [d_lo, dj, dcol]
    bp = np.asarray(b_proj, np.float32).astype(bf).reshape(1, DIM)
    in_maps = []
    for i in range(B):
        xi = np.asarray(x[i], np.float32)
        xT = xi.T  # [c, n]
        xt = np.ascontiguousarray(
            xT.reshape(CT, 128, NCH, XCH).transpose(2, 1, 0, 3)
        ).astype(bf)  # [ci, c_lo, cj, col]
        xp = xi.reshape(L, SEG, DIM).mean(axis=1)  # [l, c]
        xpool = np.ascontiguousarray(
            xp.T.reshape(CT, 128, L).transpose(1, 0, 2)
        ).astype(bf)  # [c_lo, cj, l]
        in_maps.append(
            {"xt": xt, "wkv": wkv, "xpool": xpool, "wqT": wqT, "wproj": wp, "bproj": bp}
        )
    return in_maps


_NC_CACHE = None


def kernel(x, W_qkv, W_proj, b_proj):
    global _NC_CACHE
    if _NC_CACHE is None:
        _NC_CACHE = build_nc()
    nc = _NC_CACHE
    in_maps = prep_in_maps(x, W_qkv, W_proj, b_proj)
    res = run_bass_kernel_spmd(nc, in_maps, core_ids=list(range(B)))
    out = np.stack([res.results[i]["out"] for i in range(B)], axis=0)
    return out.astype(np.float32)


# revision 29
# speedup vs baseline: 2.4966x; 1.3250x over previous
"""Trainium2 Bass kernel for AMM (landmark/Nystrom-style) attention.

Per batch element (8 total, one NeuronCore each):
    qkv  = x @ W_qkv; q,k,v = split(qkv); q /= sqrt(512)
    keys_lm = segment_mean(k, 16); vals_lm = segment_mean(v, 16)
    out  = softmax(q @ keys_lm^T) @ vals_lm @ W_proj + b_proj
    return v + out

Algebraic restructuring (exact in real arithmetic):
  - segment_mean commutes with the projections: keys_lm = pool(x) @ W_k,
    vals_lm = pool(x) @ W_v; pool(x) is computed on the host (16M adds vs
    17G device MACs) so the landmark chain starts ~2us into the kernel.
  - the q projection is never computed: scores = x @ (W_q @ keys_lm^T),
    and W_q @ keys_lm^T is only 512x256, once per core.
  - attn @ vals_lm @ W_proj -> attn @ (vals_lm @ W_proj + 1 b_proj) = VWb
    (b_proj folded via softmax rows summing to 1).
  - softmax normalization applied BEFORE the value matmul:
        attn = E * (1/den) with den broadcast to all partitions by a
    GpSimd partition_all_reduce, so the value matmul emits the final
    attention output directly.
  - THE RESIDUAL ADD IS FREE: the v projection (x @ W_v) and the value
    matmul (attn @ VWb) accumulate into the SAME PSUM tile; PSUM holds
    v + attn@VWb and the only post-op is a psum->sbuf copy (DVE, bf16)
    followed by the output DMA. No scalar_tensor_tensor, no v staging.

Engine layout: PE does all matmuls (v / landmarks / scores / out2) nearly
back-to-back; ACT does landmark copies then exps (one activation-table
switch); GpSimd does the den partition-reductions; DVE does den-sum,
reciprocal, attn scaling and the psum evictions. Output is bf16 (host
upcasts); input DMAs are chunk-major single-instruction transfers so the
HWDGE generator is never the bottleneck.

Sharding: pure data-parallel over batch B=8 across 8 cores, weights
replicated, no collectives. Host pre-transposes x per core, pre-pools the
landmark means, and casts matmul inputs to bf16 (fp32 PSUM accumulation).
"""

import sys
from contextlib import ExitStack

import numpy as np

sys.path.insert(0, "/opt/trn_rl_repo")

import concourse.bass as bass  # noqa: E402
import concourse.tile as tile  # noqa: E402
from concourse import bacc, bass_isa, mybir  # noqa: E402
from concourse.bass_utils import run_bass_kernel_spmd  # noqa: E402

import ml_dtypes  # noqa: E402

BF16 = mybir.dt.bfloat16
F32 = mybir.dt.float32
AF = mybir.ActivationFunctionType
ALU = mybir.AluOpType

B, N, DIM = 8, 4096, 512
L, SEG = 256, 16
CT = DIM // 128
MT = N // 512
XCH = 1024
NCH = N // XCH

RSCALE = float(1.0 / np.sqrt(512.0))


def build_kernel(ctx: ExitStack, tc: "tile.TileContext", out_d, xt_d, wkv_d, xpool_d, wqT_d, wproj_d, bproj_d):
    nc = tc.nc

    consts = ctx.enter_context(tc.tile_pool(name="consts", bufs=1))
    work = ctx.enter_context(tc.tile_pool(name="work", bufs=2))
    pm = ctx.enter_context(tc.tile_pool(name="pm", bufs=3, space="PSUM"))
    po = ctx.enter_context(tc.tile_pool(name="po", bufs=4, space="PSUM"))
    pd = ctx.enter_context(tc.tile_pool(name="pd", bufs=1, space="PSUM"))

    wkv = consts.tile([128, CT, 2 * DIM], BF16)  # [c_lo, cj, (k|v) columns]
    xt = consts.tile([128, CT, N], BF16)
    xpool = consts.tile([128, CT, L], BF16)
    wqT = consts.tile([128, CT, DIM], BF16)
    wproj = consts.tile([128, CT, DIM], BF16)
    bproj = consts.tile([1, DIM], BF16)
    ones_row = consts.tile([1, 128], BF16)

    # Input DMAs spread over three HWDGE queues so the first transfers
    # overlap; order within each queue matches the PE program order
    # keysT -> kw -> scores(0) -> valsT -> scores(1) -> vw -> v.
    nc.sync.dma_start(out=xpool[:, :, :], in_=xpool_d[:, :, :])
    nc.sync.dma_start(out=wkv[:, :, 0:DIM], in_=wkv_d[1])
    nc.sync.dma_start(out=wqT[:, :, :], in_=wqT_d[:, :, :])
    nc.sync.dma_start(out=xt[:, :, 0:XCH], in_=xt_d[0])
    nc.sync.dma_start(out=wkv[:, :, DIM : 2 * DIM], in_=wkv_d[0])
    nc.sync.dma_start(out=xt[:, :, XCH : 2 * XCH], in_=xt_d[1])
    nc.sync.dma_start(out=wproj[:, :, :], in_=wproj_d[:, :, :])
    nc.sync.dma_start(out=bproj[:, :], in_=bproj_d[:, :])
    for ci in range(2, NCH):
        nc.sync.dma_start(out=xt[:, :, ci * XCH : (ci + 1) * XCH], in_=xt_d[ci])

    # PE clock warm-up: the cost of a PE idle gap includes a ~3us re-ramp
    # from 0.65GHz. Run junk matmuls on a memset scratch tile while the
    # first DMAs land so the real matmuls start at full clock.
    scratch = consts.tile([128, 512], BF16)
    ones_col = consts.tile([128, 1], BF16)
    nc.vector.memset(scratch[:, :], 0.0)
    nc.vector.memset(ones_row[:, :], 1.0)
    nc.vector.memset(ones_col[:, :], 1.0)
    for w in range(8):
        wp = pm.tile([128, 512], F32, tag="mm", name="warm")
        nc.tensor.matmul(
            wp[:, :], scratch[:, 0:128], scratch[:, :], start=True, stop=True
        )

    keysT = consts.tile([128, CT, L], BF16)
    valsT = consts.tile([128, CT, L], BF16)
    kw = consts.tile([128, CT, L], BF16)
    vw = consts.tile([128, 2, DIM], BF16)

    out_ps = {}

    def v_group(t):
        # Opens the output psum tile for rows [t*128, (t+1)*128): accumulates
        # x @ W_v; the attention matmuls later close the group.
        op = po.tile([128, 512], F32, tag="out", name=f"op{t}")
        r0 = t * 128
        for cj in range(CT):
            nc.tensor.matmul(
                op[:, :],
                xt[:, cj, r0 : r0 + 128],
                wkv[:, cj, DIM : 2 * DIM],
                start=(cj == 0),
                stop=False,
                skip_group_check=True,
            )
        return op

    def lm_proj(dst, col0):
        for dj in range(CT):
            pt = pm.tile([128, L], F32, tag="mm", name="ptl")
            for cj in range(CT):
                nc.tensor.matmul(
                    pt[:, :],
                    wkv[:, cj, col0 + dj * 128 : col0 + (dj + 1) * 128],
                    xpool[:, cj, :],
                    start=(cj == 0),
                    stop=(cj == CT - 1),
                )
            nc.scalar.copy(dst[:, dj, :], pt[:, :])

    def scores_p1(mi):
        # scores -> exp. The den/normalize steps are issued separately and
        # later (PE den-row + PE broadcast + DVE recip/scale) so no engine's
        # in-order queue ever blocks the psum evictions.
        et = work.tile([128, 2, 512], BF16, tag="et", bufs=3, name="et")
        for li in range(2):
            pt = pm.tile([128, 512], F32, tag="mm", name="pts")
            for cj in range(CT):
                nc.tensor.matmul(
                    pt[:, :],
                    kw[:, cj, li * 128 : (li + 1) * 128],
                    xt[:, cj, mi * 512 : (mi + 1) * 512],
                    start=(cj == 0),
                    stop=(cj == CT - 1),
                )
            nc.scalar.activation(et[:, li, :], pt[:, :], AF.Exp, scale=RSCALE)
        return et

    def den_row(et):
        # den[1, n] = sum over all 256 landmarks of E: two N=512 matmuls with
        # the all-ones column stationary, accumulating li halves in psum.
        dp = pd.tile([128, 512], F32, tag="den", name="dp")
        for li in range(2):
            nc.tensor.matmul(
                dp[0:1, :],
                ones_col[:, :],
                et[:, li, :],
                start=(li == 0),
                stop=(li == 1),
            )
        rr = work.tile([1, 512], BF16, tag="rr", bufs=2, name="rr")
        return dp, rr

    def recip(dp, rr):
        with nc.allow_low_precision(reason="1/den tolerates bf16"):
            nc.vector.reciprocal(rr[:, :], dp[0:1, :])

    def bcast(rr):
        # Replicate 1/den to all 128 partitions with a K=1 outer-product
        # matmul: rrb = ones_row^T @ rr_row.
        rrb = pm.tile([128, 512], F32, tag="mm", name="rrb")
        nc.tensor.matmul(rrb[:, :], ones_row[:, :], rr[:, :], start=True, stop=True)
        return rrb

    def scales(et, rrb):
        attn = work.tile([128, 2, 512], BF16, tag="attn", bufs=2, name="attn")
        with nc.allow_low_precision(reason="attn weights tolerate bf16"):
            for li in range(2):
                nc.vector.tensor_tensor(
                    attn[:, li, :], et[:, li, :], rrb[:, :], ALU.mult
                )
        return attn

    # ---- pipeline fill: landmark chain interleaved with scores(0|1) --------
    lm_proj(keysT, 0)
    for cj in range(CT):
        pt = pm.tile([128, L], F32, tag="mm", name="ptk")
        for dj in range(CT):
            nc.tensor.matmul(
                pt[:, :],
                wqT[:, dj, cj * 128 : (cj + 1) * 128],
                keysT[:, dj, :],
                start=(dj == 0),
                stop=(dj == CT - 1),
            )
        nc.scalar.copy(kw[:, cj, :], pt[:, :])
    ets = {0: scores_p1(0)}
    lm_proj(valsT, DIM)
    ets[1] = scores_p1(1)
    for li in range(2):
        pt = pm.tile([128, DIM], F32, tag="mm", name="ptv")
        for dj in range(CT):
            nc.tensor.matmul(
                pt[:, :],
                valsT[:, dj, li * 128 : (li + 1) * 128],
                wproj[:, dj, :],
                start=(dj == 0),
                stop=False,
            )
        nc.tensor.matmul(pt[:, :], ones_row[:, :], bproj[:, :], start=False, stop=True)
        nc.scalar.copy(vw[:, li, :], pt[:, :])

    dp0, rr0 = den_row(ets[0])
    recip(dp0, rr0)
    for t in range(4):
        out_ps[t] = v_group(t)
    rrb0 = bcast(rr0)
    attn_q = [scales(ets.pop(0), rrb0)]

    # ---- main loop: close out tiles with attn @ VWb, evict, stream ---------
    # Block order: scores(mi+2) -> den(mi+1) [recip queued ahead of this
    # block's copies on DVE] -> out2(mi)+evictions -> v(mi+1) -> bcast(mi+1)
    # -> scales(mi+1). Each engine's in-order queue stays unblocked.
    for mi in range(MT):
        if mi + 2 < MT:
            ets[mi + 2] = scores_p1(mi + 2)
        nxt = None
        if mi + 1 < MT:
            et_nxt = ets.pop(mi + 1)
            dp, rr = den_row(et_nxt)
            recip(dp, rr)
            nxt = (et_nxt, rr)
        attn = attn_q.pop(0)
        osb = work.tile([128, 4, 512], BF16, tag="osb", bufs=2, name="osb")
        # In the last two blocks ACT has no exps left: split each eviction
        # ACT/DVE half-and-half (one Copy-table switch, off the critical
        # path) and DMA per tile so the kernel tail drains ~2x faster.
        tail = mi >= MT - 2
        r0 = mi * 512
        for t in range(4):
            op = out_ps.pop(mi * 4 + t)
            sl = slice(t * 128, (t + 1) * 128)
            for li in range(2):
                nc.tensor.matmul(
                    op[:, :],
                    attn[:, li, sl],
                    vw[:, li, :],
                    start=False,
                    stop=(li == 1),
                    skip_group_check=True,
                )
            if tail and t % 2 == 0:
                nc.scalar.copy(osb[:, t, :], op[:, :])
            else:
                nc.vector.tensor_copy(osb[:, t, :], op[:, :])
        if tail:
            for h, q in ((0, nc.sync), (1, nc.scalar)):
                q.dma_start(
                    out=out_d[r0 + h * 256 : r0 + (h + 1) * 256, :].rearrange(
                        "(t p) d -> p t d", p=128
                    ),
                    in_=osb[:, 2 * h : 2 * h + 2, :],
                )
        else:
            nc.sync.dma_start(
                out=out_d[r0 : r0 + 512, :].rearrange("(t p) d -> p t d", p=128),
                in_=osb[:, :, :],
            )
        if mi + 1 < MT:
            for t in range(4):
                out_ps[(mi + 1) * 4 + t] = v_group((mi + 1) * 4 + t)
        if nxt is not None:
            et_nxt, rr = nxt
            rrb = bcast(rr)
            attn_q.append(scales(et_nxt, rrb))


def build_nc(repeat: int = 1):
    nc = bacc.Bacc("TRN2", target_bir_lowering=False, debug=False, num_devices=8)
    xt_d = nc.declare_dram_parameter("xt", [NCH, 128, CT, XCH], BF16, isOutput=False)
    wkv_d = nc.declare_dram_parameter("wkv", [2, 128, CT, DIM], BF16, isOutput=False)
    xpool_d = nc.declare_dram_parameter("xpool", [128, CT, L], BF16, isOutput=False)
    wqT_d = nc.declare_dram_parameter("wqT", [128, CT, DIM], BF16, isOutput=False)
    wproj_d = nc.declare_dram_parameter("wproj", [128, CT, DIM], BF16, isOutput=False)
    bproj_d = nc.declare_dram_parameter("bproj", [1, DIM], BF16, isOutput=False)
    out_d = nc.declare_dram_parameter("out", [N, DIM], BF16, isOutput=True)
    aps = (
        out_d.ap(),
        xt_d.ap(),
        wkv_d.ap(),
        xpool_d.ap(),
        wqT_d.ap(),
        wproj_d.ap(),
        bproj_d.ap(),
    )
    with tile.TileContext(nc) as tc, ExitStack() as ctx:
        if repeat == 1:
            build_kernel(ctx, tc, *aps)
        else:
            with tc.For_i(0, repeat, 1):
                build_kernel(ctx, tc, *aps)
    nc.compile()
    return nc


def prep_in_maps(x, W_qkv, W_proj, b_proj):
    bf = ml_dtypes.bfloat16
    W = np.asarray(W_qkv, np.float32)
    kv = W[:, DIM:].reshape(CT, 128, 2, DIM)  # [cj, c_lo, (k,v), col]
    wkv = np.ascontiguousarray(
        np.stack([kv[:, :, 1, :], kv[:, :, 0, :]], axis=0).transpose(0, 2, 1, 3)
    ).astype(bf)  # [v|k, c_lo, cj, col]
    wqT = np.ascontiguousarray(
        W[:, :DIM].T.reshape(CT, 128, DIM).transpose(1, 0, 2)
    ).astype(bf)  # [d_lo, dj, c]
    wp = np.ascontiguousarray(
        np.asarray(W_proj, np.float32).reshape(CT, 128, DIM).transpose(1, 0, 2)
    ).astype(bf)  # [d_lo, dj, dcol]
    bp = np.asarray(b_proj, np.float32).astype(bf).reshape(1, DIM)
    in_maps = []
    for i in range(B):
        xi = np.asarray(x[i], np.float32)
        xT = xi.T  # [c, n]
        xt = np.ascontiguousarray(
            xT.reshape(CT, 128, NCH, XCH).transpose(2, 1, 0, 3)
        ).astype(bf)  # [ci, c_lo, cj, col]
        xp = xi.reshape(L, SEG, DIM).mean(axis=1)  # [l, c]
        xpool = np.ascontiguousarray(
            xp.T.reshape(CT, 128, L).transpose(1, 0, 2)
        ).astype(bf)  # [c_lo, cj, l]
        in_maps.append(
            {"xt": xt, "wkv": wkv, "xpool": xpool, "wqT": wqT, "wproj": wp, "bproj": bp}
        )
    return in_maps


_NC_CACHE = None


def kernel(x, W_qkv, W_proj, b_proj):
    global _NC_CACHE
    if _NC_CACHE is None:
        _NC_CACHE = build_nc()
    nc = _NC_CACHE
    in_maps = prep_in_maps(x, W_qkv, W_proj, b_proj)
    res = run_bass_kernel_spmd(nc, in_maps, core_ids=list(range(B)))
    out = np.stack([res.results[i]["out"] for i in range(B)], axis=0)
    return out.astype(np.float32)
